# revision 14
# baseline (speedup 1.0000x reference)
"""Distributed Trainium2 kernel for nn_Attention_15710990369355.

Attention with QK-layernorm, sharded over 8 NeuronCores, collective-free:
  core c -> batch b = c // 4, head-group hg = c % 4 (4 of 16 heads).
  qkv weights column-sharded per head group; attention fully local;
  proj ROW-sharded (each core contracts only its own 256 head-rows) so
  each core emits a transposed partial out^T [1024, 2048]; the host sums
  the 4 partials per batch during unsharding (collectives on this stack
  cost ~60-100us each, far more than the host-side reduce).

Compute dtype: bf16 operands with fp32 PSUM accumulation; partial
outputs in fp16 (range is tiny, keeps output DMA at half cost).

v2 schedule (TimelineSim-driven rebalance of the v1 kernel):
  - DMA order: tiny gamma/beta first, then wqk chunk 0 + xt slice 0 so
    the first QKV matmul starts ~8us earlier; wv/wp/pb ride later.
  - qkraw evacuation alternates ACT/DVE (was ACT-only) -- ACT's exp
    stream is the global floor (~147us busy), everything else must stay
    off it; LN square+segmented-reduce go to the idle GPSIMD.
  - qhat normalize is ONE broadcast-AP multiply per tile (rstd bf16,
    free-dim stride-0) instead of 8 per-group tensor_scalars.
  - attention: S^T+exp emitted BEFORE the AV chunk each iteration so PE
    always has the next st ready ahead of ACT; proj is per-query-block
    (8 [128,256] units) spread one-per-odd-m through the NEXT qb's loop
    instead of a 16-matmul burst that starved the exp stream.
  - av accumulator split av01/av23 (1 PSUM bank each) so the qb+1 AV
    only waits on the half-norm that consumed its bank.
  - proj bias-add evacuation runs on DVE early, on ACT for the last
    query blocks (ACT is idle in the tail, DVE is busy with norms).

PSUM budget (8 banks): st 2x[128,1024] = 4, av01/av23 2x[65,512] = 2,
everything else (pq/transpose/vq/bc/pp) 2x[128,512] = 2.
"""

import numpy as np
import ml_dtypes

import concourse.bass as bass
import concourse.mybir as mybir
import concourse.tile as tile
from concourse import bacc
from concourse import hw_specs as _hw_specs
from concourse.bass_utils import run_bass_kernel_spmd
from concourse.masks import make_identity

# The act-table chooser picks the FIRST set containing each function
# (Exp -> exp_and_others, Ln -> natural_log_...), so an Exp/Ln mix thrashes
# ~1.3us table loads + drains. Restrict Exp/Ln to the combined
# natural_log_exp set (index preserved -> correct runtime table id) so the
# whole kernel runs off one table load.
_orig_gat = _hw_specs.get_activation_tables


def _gat_combined(arch):
    out = {}
    for name, fns in _orig_gat(arch).items():
        if name != "natural_log_exp_and_others":
            fns = {
                f for f in fns
                if f not in (mybir.ActivationFunctionType.Exp,
                             mybir.ActivationFunctionType.Ln,
                             mybir.ActivationFunctionType.Copy,
                             mybir.ActivationFunctionType.Identity,
                             mybir.ActivationFunctionType.Square)
            }
        out[name] = fns
    return out


_hw_specs.get_activation_tables = _gat_combined
bacc.get_activation_tables = _gat_combined

BF16 = mybir.dt.bfloat16
F16 = mybir.dt.float16
F32 = mybir.dt.float32
AF = mybir.ActivationFunctionType
ALU = mybir.AluOpType

B, N, C, H = 2, 2048, 1024, 16
Dh = C // H              # 64
HPC = 4                  # heads per core
NT = N // 128            # 16 row tiles
CK = C // 128            # 8 contraction chunks of the C dim
EPS = 1e-6
SCALE = Dh ** -0.5       # 0.125
NQB = 256                # query block size in attention
NQBS = N // NQB          # 8 query blocks
NCORES = 8

nbf = ml_dtypes.bfloat16


def _bcast_last(ap: bass.AP, n: int) -> bass.AP:
    """Append a stride-0 broadcast dim of size n to an AP."""
    return bass.AP(ap.tensor, ap.offset, list(ap.ap) + [[0, n]])


def build(reps=1, trivial_gb=True):
    nc = bacc.Bacc(
        "TRN2",
        target_bir_lowering=False,
        debug=False,
        enable_asserts=False,
        num_devices=NCORES,
    )

    # ---- dram parameters (per-core shards supplied by host) ----
    xt_d = nc.dram_tensor("xt", [C, N], BF16, kind="ExternalInput").ap()
    wqk_d = nc.dram_tensor("wqk", [C, 512], BF16, kind="ExternalInput").ap()
    wv_d = nc.dram_tensor("wv", [C, 256], BF16, kind="ExternalInput").ap()
    wp_d = nc.dram_tensor("wp", [256, C], BF16, kind="ExternalInput").ap()
    pb_d = nc.dram_tensor("pb", [128, 8], F32, kind="ExternalInput").ap()
    qgb_d = nc.dram_tensor("qgb", [128, 2], F32, kind="ExternalInput").ap()
    kgb_d = nc.dram_tensor("kgb", [128, 2], F32, kind="ExternalInput").ap()
    out_d = nc.dram_tensor("out", [C, N], F16, kind="ExternalOutput").ap()

    with tile.TileContext(nc) as tc:
        with (
            tc.tile_pool(name="singles", bufs=1) as singles,
            tc.tile_pool(name="psum_big", bufs=2, space="PSUM") as psum_big,
            tc.tile_pool(name="psum_av", bufs=2, space="PSUM") as psum_av,
            tc.tile_pool(name="psum_one", bufs=2, space="PSUM") as psum_one,
            tc.tile_pool(name="work", bufs=3) as work,
            tc.tile_pool(name="pt_pool", bufs=34) as pt_pool,
            tc.tile_pool(name="small", bufs=4) as small,
            tc.tile_pool(name="outp", bufs=4) as outp,
        ):
            for rep in range(reps):
                _emit(nc, tc, locals(), trivial_gb)

    nc.finalize()
    return nc


def _emit(nc, tc, env, trivial_gb):
    singles = env["singles"]
    psum_big = env["psum_big"]
    psum_av = env["psum_av"]
    psum_one = env["psum_one"]
    work = env["work"]
    pt_pool = env["pt_pool"]
    small = env["small"]
    outp = env["outp"]
    xt_d, wqk_d, wv_d, wp_d, pb_d, qgb_d, kgb_d, out_d = (
        env["xt_d"], env["wqk_d"], env["wv_d"], env["wp_d"],
        env["pb_d"], env["qgb_d"], env["kgb_d"], env["out_d"],
    )

    # ---------------- loads, ordered for earliest compute start -----------
    # exactly what the opening QKV matmuls consume first: wqk chunk 0 +
    # xt slice 0; the tiny LN params ride after slice 1 (needed ~t=15us).
    wqk_s = singles.tile([128, CK, 512], BF16, name="wqk_s", tag="wqk_s")
    wqk_r = wqk_d.rearrange("(a p) n -> p a n", p=128)
    nc.sync.dma_start(out=wqk_s[:, 0:1, :], in_=wqk_r[:, 0:1, :])

    xt_s = singles.tile([128, CK, N], BF16, name="xt_s", tag="xt_s")
    xt_r = xt_d.rearrange("(a p) n -> p a n", p=128)
    nc.sync.dma_start(out=xt_s[:, :, 0:256], in_=xt_r[:, :, 0:256])
    nc.sync.dma_start(out=wqk_s[:, 1:4, :], in_=wqk_r[:, 1:4, :])
    nc.sync.dma_start(out=xt_s[:, :, 256:512], in_=xt_r[:, :, 256:512])
    qgb_s = singles.tile([128, 2], F32, name="qgb_s", tag="qgb_s")
    nc.sync.dma_start(out=qgb_s, in_=qgb_d)
    kgb_s = singles.tile([128, 2], F32, name="kgb_s", tag="kgb_s")
    nc.sync.dma_start(out=kgb_s, in_=kgb_d)
    nc.sync.dma_start(out=wqk_s[:, 4:8, :], in_=wqk_r[:, 4:8, :])
    for i in range(2, 8):
        sl = bass.ts(i, N // 8)
        nc.sync.dma_start(out=xt_s[:, :, sl], in_=xt_r[:, :, sl])

    wv_s = singles.tile([128, CK, 256], BF16, name="wv_s", tag="wv_s")
    wv_r = wv_d.rearrange("(a p) n -> p a n", p=128)
    nc.sync.dma_start(out=wv_s[:, 0:4, :], in_=wv_r[:, 0:4, :])
    nc.sync.dma_start(out=wv_s[:, 4:8, :], in_=wv_r[:, 4:8, :])
    wp_s = singles.tile([128, 2, C], BF16, name="wp_s", tag="wp_s")
    nc.sync.dma_start(out=wp_s, in_=wp_d.rearrange("(a p) n -> p a n", p=128))
    pb4_s = singles.tile([128, 8], F32, name="pb4_s", tag="pb4_s")
    nc.sync.dma_start(out=pb4_s, in_=pb_d)

    ident = singles.tile([128, 128], BF16, name="ident", tag="ident")
    make_identity(nc, ident)
    # 1/s partition-broadcast selector: K=1 ones row -> 64 output rows
    e1_64 = singles.tile([1, 64], BF16, name="e1_64", tag="e1_64")
    nc.vector.memset(e1_64, 1.0)

    # PE warmup: ~24 junk matmuls on the identity while the first DMAs
    # land, so the HAM clock-gate releases (1.2->2.4 GHz takes ~3.4us of
    # sustained PE activity) before the first real QKV matmul.
    warm = psum_one.tile([128, 128], F32, name="warm", tag="one")
    for _ in range(24):
        nc.tensor.matmul(warm, ident, ident, start=True, stop=True)

    # ---------------- persistent activations ----------------
    # V with a ones-column appended (col 64): AV matmuls with this as the
    # stationary produce the softmax denominator in output row 64 for free.
    v65 = singles.tile([128, NT, HPC, Dh + 1], BF16, name="v65", tag="v65")
    nc.vector.memset(v65[:, :, :, Dh:Dh + 1], 1.0)
    # q̂^T / k̂^T in head-pairs: [128=(2 heads x 64d), pair, n]
    # pair 0,1 = q head pairs (0,1),(2,3); pair 2,3 = k head pairs
    qkT = singles.tile([128, 4, N], BF16, name="qkT", tag="qkT")
    # attention output, transposed: chunk 0 = heads 0,1; chunk 1 = heads 2,3
    ot_s = singles.tile([128, 2, N], BF16, name="ot_s", tag="ot_s")
    qkraw_all = singles.tile([128, NT, 512], BF16, name="qkraw_all", tag="qkraw_all")
    ssq_all = singles.tile([128, NT, 8], F32, name="ssq_all", tag="ssq_all")
    rstd_all = singles.tile([128, NT, 8], BF16, name="rstd_all", tag="rstd_all")

    # ---------------- attention helpers ----------------
    # PSUM layout of st [128, 1024]: head h at cols [256h, 256h+256);
    # pt column order [A | C | B | D]: head h lives at pt slot PTSLOT[h].
    PTSLOT = (0, 2, 1, 3)

    av_tiles = {}
    pts = {}

    def attn_alloc(qb):
        # av halves: heads 0,1 in av01 (1 bank), heads 2,3 in av23 (1 bank).
        # row 64 = softmax denominator (from the v65 ones column). Separate
        # banks let qb+1's AV start as soon as each half-norm has read its
        # bank, instead of waiting for the full norm.
        av_tiles[(qb, 0)] = psum_av.tile([65, 512], F32, name=f"av01_{qb}", tag="av")
        av_tiles[(qb, 1)] = psum_av.tile([65, 512], F32, name=f"av23_{qb}", tag="av")

    def s_exp_chunk(qb, m):
        # S^T + exp only; pt is buffered in SBUF so the ACT exp stream
        # never waits on AV accumulator (PSUM) availability.
        qcols = bass.ts(qb, NQB)
        kcols = bass.ts(m, 128)
        st = psum_big.tile([128, 1024], F32, name=f"st_{qb}_{m}", tag="big")
        # row-packed pairs (A,B) then (C,D); column layout [A | C | B | D]
        nc.tensor.matmul(
            st[:, 0:NQB], qkT[0:64, 2, kcols], qkT[0:64, 0, qcols],
            start=True, stop=True,
        )
        nc.tensor.matmul(
            st[:, 512:512 + NQB], qkT[64:128, 2, kcols], qkT[64:128, 0, qcols],
            start=True, stop=True,
        )
        nc.tensor.matmul(
            st[:, NQB:512], qkT[0:64, 3, kcols], qkT[0:64, 1, qcols],
            start=True, stop=True,
        )
        nc.tensor.matmul(
            st[:, 512 + NQB:1024], qkT[64:128, 3, kcols], qkT[64:128, 1, qcols],
            start=True, stop=True,
        )
        pt = pt_pool.tile([128, 4, NQB], BF16, name=f"pt_{qb}_{m}", tag="pt")
        nc.scalar.activation(
            out=pt.rearrange("p a n -> p (a n)"), in_=st, func=AF.Exp,
            scale=SCALE,
        )
        pts[(qb, m)] = pt

    def av_chunk(qb, m, heads=range(HPC)):
        # AV, full-array mode (M=65): denominator rides along in row 64.
        # start=True clears has_written for the WHOLE bank; the first head
        # of each bank starts the group, the second inherits cleared bits at
        # m==0 (clear bit -> overwrite) which is the accumulate-start
        # semantic.
        pt = pts[(qb, m)]
        for h in heads:
            av = av_tiles[(qb, h // 2)]
            nc.tensor.matmul(
                av[:, bass.ts(h % 2, NQB)], v65[:, m, h, :], pt[:, PTSLOT[h], :],
                start=(m == 0 and h % 2 == 0), stop=(m == NT - 1),
                skip_group_check=True,
            )
        if heads == range(HPC) or heads == (2, 3):
            pts.pop((qb, m))

    def v_tile(m):
        # V projection for key-tile m, off the phase-1 critical path: runs
        # in the attention phase's PE slack; evacuation on DVE (not the
        # bottleneck ACT engine).
        mcols = bass.ts(m, 128)
        vq = psum_one.tile([128, 256], F32, name=f"vq_{m}", tag="one")
        for c in range(CK):
            nc.tensor.matmul(
                vq, xt_s[:, c, mcols], wv_s[:, c, :],
                start=(c == 0), stop=(c == CK - 1),
            )
        nc.vector.tensor_copy(
            out=v65[:, m, :, 0:Dh],
            in_=vq.rearrange("p (h d) -> p h d", h=HPC),
        )

    def attn_norm_half(qb, half):
        # normalize heads (2*half, 2*half+1) from their av bank; frees that
        # bank for qb+1's AV as soon as the reads complete.
        av = av_tiles.pop((qb, half))
        qcols = bass.ts(qb, NQB)
        rs = small.tile([1, 512], BF16, name=f"rs_{qb}_{half}", tag="rs")
        with nc.allow_low_precision(reason="1/s to bf16 for PE broadcast"):
            nc.vector.reciprocal(out=rs, in_=av[64:65, :])
        bc = psum_one.tile([64, 512], F32, name=f"bc_{qb}_{half}", tag="one")
        nc.tensor.matmul(bc, e1_64, rs, start=True, stop=True)
        rsb = small.tile([64, 512], BF16, name=f"rsb_{qb}_{half}", tag="rsb")
        nc.vector.tensor_copy(out=rsb, in_=bc)
        for h in (2 * half, 2 * half + 1):
            hl = bass.ts(h % 2, NQB)
            nc.vector.tensor_mul(
                ot_s[64 * (h % 2):64 * (h % 2) + 64, h // 2, qcols],
                av[0:64, hl], rsb[:, hl],
            )

    def attn_norm(qb):
        attn_norm_half(qb, 0)
        attn_norm_half(qb, 1)

    def proj_unit(qb, ot, on_act):
        # one [128 out-chans, 256 q] slice of the row-sharded proj partial.
        # Bias = pb/4 on every core (exact for the 4-way partial sum).
        qcols = bass.ts(qb, NQB)
        occols = bass.ts(ot, 128)
        pp = psum_one.tile([128, 256], F32, name=f"pp_{qb}_{ot}", tag="one")
        nc.tensor.matmul(
            pp, wp_s[:, 0, occols], ot_s[:, 0, qcols],
            start=True, stop=False,
        )
        nc.tensor.matmul(
            pp, wp_s[:, 1, occols], ot_s[:, 1, qcols],
            start=False, stop=True,
        )
        o_t = outp.tile([128, 256], F16, name=f"o_{qb}_{ot}", tag="o")
        if on_act:
            # tail query blocks: ACT has no exps left, DVE is doing norms
            nc.scalar.activation(
                out=o_t, in_=pp, func=AF.Identity,
                bias=pb4_s[:, ot:ot + 1], scale=1.0,
            )
        else:
            nc.vector.tensor_scalar(
                out=o_t, in0=pp, scalar1=pb4_s[:, ot:ot + 1], scalar2=None,
                op0=ALU.add,
            )
        nc.sync.dma_start(out=out_d[occols, qcols], in_=o_t)

    # ---- phase 1a: Q|K only (V is deferred into the attention phase) ----
    # Per-tile pipeline with explicit lags so no engine's in-order queue
    # blocks on a cross-engine chain:
    #   iter t: QKV(t) [PE] -> evac(t) [ACT/DVE] -> sq(t) [Pool]
    #           reduce(t-1) [DVE]   (1-tile lag hides Pool latency)
    #           group stats when the group's last reduce is in
    #           finish_tile(tt) (qhat/transpose/qkT) 2+ tiles behind
    #           2 interleaved S+exp chunks once their qkT tiles exist
    _LN_GROUPS = {2: (0, 2), 4: (2, 2), 8: (4, 4), 12: (8, 4), 16: (12, 4)}
    _FINISH_AT = {3: [0], 4: [1], 5: [2], 6: [3], 9: [4], 10: [5],
                  11: [6], 12: [7], 13: [8], 14: [9], 15: [10]}
    _finished = set()

    def ln_stats(ts0, nts):
        gsl = slice(ts0, ts0 + nts)
        sc = small.tile([128, 8 * nts], F32, name=f"sc_{ts0}", tag="sc")
        nc.vector.tensor_scalar(
            out=sc, in0=ssq_all[:, gsl, :].rearrange("p a b -> p (a b)"),
            scalar1=1.0 / Dh, scalar2=EPS, op0=ALU.mult, op1=ALU.add,
        )
        # rstd = exp(-0.5*ln(var)): stays in the natural_log_exp table
        # set shared with the attention exps (no ACT table swaps).
        lnv = small.tile([128, 8 * nts], F32, name=f"lnv_{ts0}", tag="lnv")
        nc.scalar.activation(out=lnv, in_=sc, func=AF.Ln)
        with nc.allow_low_precision(reason="rstd bf16: 2e-3 extra noise"):
            nc.scalar.activation(
                out=rstd_all[:, gsl, :].rearrange("p a b -> p (a b)"),
                in_=lnv, func=AF.Exp, scale=-0.5,
            )

    def finish_tile(tt):
        tcols = bass.ts(tt, 128)
        qhat = work.tile([128, 512], BF16, name=f"qhat_{tt}", tag="qhat")
        # single broadcast multiply: rstd [128, 8] stride-0 over Dh
        nc.vector.tensor_mul(
            qhat.rearrange("p (g d) -> p g d", g=8),
            qkraw_all[:, tt, :].rearrange("p (g d) -> p g d", g=8),
            _bcast_last(rstd_all[:, tt, :], Dh),
        )
        pt_ps = psum_one.tile([128, 512], BF16, name=f"tp_{tt}", tag="one")
        for p in range(4):
            nc.tensor.transpose(
                pt_ps[:, bass.ts(p, 128)], qhat[:, bass.ts(p, 128)], ident
            )
        if trivial_gb:
            nc.vector.tensor_copy(
                out=qkT[:, 0:4, tcols],
                in_=pt_ps.rearrange("p (a n) -> p a n", a=4),
            )
        else:
            nc.vector.tensor_scalar(
                out=qkT[:, 0:2, tcols],
                in0=pt_ps[:, 0:256].rearrange("p (a n) -> p a n", a=2),
                scalar1=qgb_s[:, 0:1], scalar2=qgb_s[:, 1:2],
                op0=ALU.mult, op1=ALU.add,
            )
            nc.vector.tensor_scalar(
                out=qkT[:, 2:4, tcols],
                in0=pt_ps[:, 256:512].rearrange("p (a n) -> p a n", a=2),
                scalar1=kgb_s[:, 0:1], scalar2=kgb_s[:, 1:2],
                op0=ALU.mult, op1=ALU.add,
            )
        _finished.add(tt)

    # interleaved exps, in feed order; (qb, m) needs qkT q-tiles (qb0 ->
    # 0,1; qb1 -> 2,3) and k-tile m finished.
    _exp_feed = [(qb, m) for m in range(NT) for qb in (0, 1)]

    def emit_ready_exps(limit):
        n = 0
        while _exp_feed and n < limit:
            qb, m = _exp_feed[0]
            if not ({2 * qb, 2 * qb + 1, m} <= _finished):
                break
            _exp_feed.pop(0)
            s_exp_chunk(qb, m)
            n += 1

    sqs = {}
    for t in range(NT):
        ncols = bass.ts(t, 128)
        # pq borrows the av pool (idle until phase 1b) so phase-1a's
        # transposes never contend with QKV accumulation in psum_one.
        pq = psum_av.tile([128, 512], F32, name=f"pq_{t}", tag="av")
        for c in range(CK):
            nc.tensor.matmul(
                pq, xt_s[:, c, ncols], wqk_s[:, c, :],
                start=(c == 0), stop=(c == CK - 1),
            )
        # raw q|k evac (bf16), alternating ACT/DVE to balance phase-1 load
        if t % 2 == 0:
            nc.scalar.copy(out=qkraw_all[:, t, :], in_=pq)
        else:
            nc.vector.tensor_copy(out=qkraw_all[:, t, :], in_=pq)
        # LN square on the otherwise-idle GPSIMD: the host pre-centers
        # wq/wk per head so mean(q)=mean(k)=0 exactly -- var = E[q^2].
        sq = work.tile([128, 512], BF16, name=f"sq_{t}", tag="sq")
        nc.gpsimd.tensor_mul(sq, qkraw_all[:, t, :], qkraw_all[:, t, :])
        sqs[t] = sq
        if t >= 1:
            nc.vector.reduce_sum(
                out=ssq_all[:, t - 1, :],
                in_=sqs.pop(t - 1).rearrange("p (g d) -> p g d", g=8),
                axis=mybir.AxisListType.X,
            )
        if t in _LN_GROUPS:
            ln_stats(*_LN_GROUPS[t])
        for tt in _FINISH_AT.get(t, []):
            finish_tile(tt)
        if t >= 5:
            emit_ready_exps(2)

    # ---- phase 1b: last reduces/tiles + V projection + qb0/1/2 catchup --
    nc.vector.reduce_sum(
        out=ssq_all[:, NT - 1, :],
        in_=sqs.pop(NT - 1).rearrange("p (g d) -> p g d", g=8),
        axis=mybir.AxisListType.X,
    )
    ln_stats(*_LN_GROUPS[16])
    attn_alloc(0)
    _late = [tt for tt in range(NT) if tt not in _finished]
    for m in range(NT):
        v_tile(m)
        if m < len(_late):
            finish_tile(_late[m])
        emit_ready_exps(2)
        s_exp_chunk(2, m)
        if m < NT - 1:
            av_chunk(0, m)
        else:
            # half-split finale: norm half 0 overlaps the h2/h3 matmuls
            av_chunk(0, m, heads=(0, 1))
            attn_norm_half(0, 0)
            av_chunk(0, m, heads=(2, 3))
            attn_norm_half(0, 1)
    emit_ready_exps(99)
    assert not _exp_feed, f"unfed exps: {_exp_feed}"

    # ---------------- steady state ----------------
    # lag-2 exp feed (qb+2's S/exp inside qb's AV loop) keeps the exp
    # stream dense; the previous qb's proj spreads one unit per odd m so
    # PE never does a 16-matmul burst that starves ACT.
    for qb in range(1, NQBS - 2):
        attn_alloc(qb)
        for m in range(NT):
            if qb + 2 < NQBS:
                s_exp_chunk(qb + 2, m)
            if m < NT - 1:
                av_chunk(qb, m)
                if m % 2 == 1:
                    proj_unit(qb - 1, m // 2, on_act=(qb - 1 >= 5))
            else:
                # half-split finale: norm half 0 (recip chain on DVE)
                # overlaps the h2/h3 AV matmuls + last proj unit on PE
                av_chunk(qb, m, heads=(0, 1))
                attn_norm_half(qb, 0)
                av_chunk(qb, m, heads=(2, 3))
                proj_unit(qb - 1, 7, on_act=(qb - 1 >= 5))
                attn_norm_half(qb, 1)

    # ---------------- tail: qb6 + qb7 AV merged, qb7 lag-2 ----------------
    # After the last exp (qb7, emitted in qb5's loop) the st slots in
    # psum_big are dead, so qb7's av halves borrow them: both final query
    # blocks accumulate in ONE staggered pass instead of two serial
    # PE-only loops; qb6's norms + proj overlap qb7's AV tail.
    q6, q7 = NQBS - 2, NQBS - 1
    attn_alloc(q6)
    av_tiles[(q7, 0)] = psum_big.tile([65, 512], F32, name="av01_7", tag="big")
    av_tiles[(q7, 1)] = psum_big.tile([65, 512], F32, name="av23_7", tag="big")
    LAG = 2
    for i in range(NT + LAG):
        if i < NT - 1:
            av_chunk(q6, i)
        elif i == NT - 1:
            av_chunk(q6, i, heads=(0, 1))
            attn_norm_half(q6, 0)
            av_chunk(q6, i, heads=(2, 3))
        if i >= LAG:
            m7 = i - LAG
            if m7 < NT - 1:
                av_chunk(q7, m7)
            else:
                av_chunk(q7, m7, heads=(0, 1))
                attn_norm_half(q7, 0)
                av_chunk(q7, m7, heads=(2, 3))
                attn_norm_half(q7, 1)
        if i == NT - 1:
            attn_norm_half(q6, 1)
        if i % 2 == 1 and i < NT:
            proj_unit(NQBS - 3, i // 2, on_act=True)
    # final projections, 512 wide (qb6+qb7 columns together): half the
    # per-unit evac/DMA overhead of the spread 256-wide units
    qcols2 = slice(q6 * NQB, (q7 + 1) * NQB)
    for ot in range(8):
        occols = bass.ts(ot, 128)
        pp = psum_one.tile([128, 512], F32, name=f"pp2_{ot}", tag="one")
        nc.tensor.matmul(
            pp, wp_s[:, 0, occols], ot_s[:, 0, qcols2],
            start=True, stop=False,
        )
        nc.tensor.matmul(
            pp, wp_s[:, 1, occols], ot_s[:, 1, qcols2],
            start=False, stop=True,
        )
        o_t = outp.tile([128, 512], F16, name=f"o2_{ot}", tag="o")
        if ot % 2 == 0:
            nc.scalar.activation(
                out=o_t, in_=pp, func=AF.Identity,
                bias=pb4_s[:, ot:ot + 1], scale=1.0,
            )
        else:
            nc.vector.tensor_scalar(
                out=o_t, in0=pp, scalar1=pb4_s[:, ot:ot + 1], scalar2=None,
                op0=ALU.add,
            )
        nc.sync.dma_start(out=out_d[occols, qcols2], in_=o_t)

_CACHE = {}


def _shard_inputs(x, qkv_w, q_gamma, q_beta, k_gamma, k_beta, proj_w, proj_b):
    w = np.asarray(qkv_w, np.float32).reshape(C, 3, H, Dh)
    pw = np.asarray(proj_w, np.float32)
    pb = np.asarray(proj_b, np.float32)
    x = np.asarray(x, np.float32)

    def gb(gamma, beta):
        g2 = np.concatenate([np.asarray(gamma, np.float32)] * 2)
        b2 = np.concatenate([np.asarray(beta, np.float32)] * 2)
        return np.ascontiguousarray(np.stack([g2, b2], axis=1))

    qgb = gb(q_gamma, q_beta)
    kgb = gb(k_gamma, k_beta)

    # Pre-center wq/wk per head: LN's mean-subtract is linear in the
    # weights, so subtracting each head's column mean makes mean(q)=0
    # exactly -- the kernel then only needs E[q^2] for the variance.
    wc = w.copy()
    wc[:, 0:2] -= wc[:, 0:2].mean(axis=3, keepdims=True)

    in_maps = []
    for core in range(NCORES):
        b, hg = divmod(core, 4)
        hs = slice(4 * hg, 4 * hg + 4)
        wq = wc[:, 0, hs, :].reshape(C, 256)
        wk = wc[:, 1, hs, :].reshape(C, 256)
        wv = wc[:, 2, hs, :].reshape(C, 256)
        wqk = np.concatenate([wq, wk], axis=1)
        in_maps.append({
            "xt": np.ascontiguousarray(x[b].T).astype(nbf),
            "wqk": np.ascontiguousarray(wqk).astype(nbf),
            "wv": np.ascontiguousarray(wv).astype(nbf),
            "wp": np.ascontiguousarray(pw[256 * hg:256 * (hg + 1), :]).astype(nbf),
            "pb": np.ascontiguousarray((pb / 4.0).reshape(8, 128).T).astype(np.float32),
            "qgb": qgb,
            "kgb": kgb,
        })
    return in_maps


def _gb_trivial(inputs):
    return (
        np.allclose(np.asarray(inputs["q_gamma"]), 1.0)
        and np.allclose(np.asarray(inputs["k_gamma"]), 1.0)
        and np.allclose(np.asarray(inputs["q_beta"]), 0.0)
        and np.allclose(np.asarray(inputs["k_beta"]), 0.0)
    )


def run(inputs, trace=False, **kw):
    trivial = _gb_trivial(inputs)
    key = ("nc", trivial)
    if key not in _CACHE:
        _CACHE[key] = build(trivial_gb=trivial)
    nc = _CACHE[key]
    in_maps = _shard_inputs(**inputs)
    try:
        res = run_bass_kernel_spmd(
            nc, in_maps, core_ids=list(range(NCORES)), trace=trace, **kw
        )
    except ModuleNotFoundError:
        # axon NTFF profile hook not shipped in this container; fall back to
        # an untraced run rather than crashing when BASS_TRACE is set.
        import os
        os.environ["BASS_NEVER_TRACE"] = "1"
        res = run_bass_kernel_spmd(
            nc, in_maps, core_ids=list(range(NCORES)), trace=False, **kw
        )
    out = np.empty((B, N, C), np.float32)
    for b in range(B):
        acc = np.zeros((C, N), np.float32)
        for hg in range(4):
            acc += res.results[4 * b + hg]["out"].astype(np.float32)
        out[b] = acc.T
    return out, res


def kernel(**inputs) -> np.ndarray:
    out, _ = run(inputs)
    return out


# ---------------------------------------------------------------------------
# timing apparatus (dev only): the container has no NTFF profiling, so device
# time is estimated from wall-clock slopes of async-pipelined executions,
# differencing reps=1 vs reps=K NEFFs (per-call overhead cancels).
# ---------------------------------------------------------------------------

def _make_runner(nc, in_maps):
    import jax
    import jax.numpy as jnp
    from jax.experimental.shard_map import shard_map
    from jax.sharding import Mesh, NamedSharding, PartitionSpec
    import concourse.mybir as mybir_
    from concourse import bass2jax

    bass2jax.install_neuronx_cc_hook()

    in_names, out_names, out_avals = [], [], []
    partition_name = (
        nc.partition_id_tensor.name if nc.partition_id_tensor else None
    )
    for alloc in nc.m.functions[0].allocations:
        if not isinstance(alloc, mybir_.MemoryLocationSet):
            continue
        name = alloc.memorylocations[0].name
        if alloc.kind == "ExternalInput":
            if name != partition_name:
                in_names.append(name)
        elif alloc.kind == "ExternalOutput":
            out_names.append(name)
            out_avals.append(
                jax.core.ShapedArray(
                    tuple(alloc.tensor_shape), mybir_.dt.np(alloc.dtype)
                )
            )
    n_params = len(in_names)
    all_in_names = in_names + out_names
    if partition_name is not None:
        all_in_names.append(partition_name)

    def _body(*args):
        operands = list(args)
        if partition_name is not None:
            operands.append(bass2jax.partition_id_tensor())
        outs = bass2jax._bass_exec_p.bind(
            *operands,
            out_avals=tuple(out_avals),
            in_names=tuple(all_in_names),
            out_names=tuple(out_names),
            lowering_input_output_aliases=(),
            sim_require_finite=True,
            sim_require_nnan=True,
            nc=nc,
        )
        return tuple(outs)

    devices = jax.devices()[:NCORES]
    mesh = Mesh(np.asarray(devices), ("core",))
    sharded = jax.jit(
        shard_map(
            _body, mesh=mesh,
            in_specs=(PartitionSpec("core"),) * (n_params + len(out_names)),
            out_specs=(PartitionSpec("core"),) * len(out_names),
            check_rep=False,
        ),
        keep_unused=True,
    )
    sh = NamedSharding(mesh, PartitionSpec("core"))
    concat_in = [
        jax.device_put(
            np.concatenate([np.asarray(in_maps[c][nm]) for c in range(NCORES)], 0),
            sh,
        )
        for nm in in_names
    ]
    # zero "output seed" params: not donated (kernel writes every output
    # element), so the same device buffers are reused every call.
    concat_in += [
        jax.device_put(
            np.zeros((NCORES * a.shape[0],) + tuple(a.shape[1:]), a.dtype), sh
        )
        for a in out_avals
    ]

    def call_async():
        return sharded(*concat_in)

    def call():
        out = call_async()
        jax.block_until_ready(out)
        return out

    call.call_async = call_async
    return call


def measure_slope(reps=1, iters=10, lo_m=4, hi_m=24):
    """Median wall-clock slope (seconds per async-dispatched execution) of a
    build(reps=reps) executable. Run in its OWN process: two bass
    executables in one process desync the axon terminal."""
    import time
    import jax
    import sys as _sys
    if "/root/problem" not in _sys.path:
        _sys.path.insert(0, "/root/problem")
    import reference
    cpu = jax.devices("cpu")[0]
    with jax.default_device(cpu):
        inputs = {k: np.asarray(v) for k, v in reference.setup_inputs().items()}
    in_maps = _shard_inputs(**inputs)

    call = _make_runner(build(reps=reps, trivial_gb=_gb_trivial(inputs)), in_maps)
    call()  # warm up (compile + first exec)
    call()

    def pipeline(m):
        t0 = time.perf_counter()
        outs = [call.call_async() for _ in range(m)]
        jax.block_until_ready(outs)
        return time.perf_counter() - t0

    # Quietest-observed wall time for each pipeline depth (min over trials
    # rejects ambient-load spikes), then difference the minima.
    tls, ths = [], []
    for _ in range(iters):
        tls.append(pipeline(lo_m))
        ths.append(pipeline(hi_m))
    slope = (min(ths) - min(tls)) / (hi_m - lo_m)
    slopes = sorted((th - tl) / (hi_m - lo_m) for tl, th in zip(tls, ths))
    print(f"reps={reps} slopes us: {[f'{s * 1e6:.1f}' for s in slopes]}"
          f" min-diff {slope * 1e6:.1f}")
    return slope
